# revision 18
# baseline (speedup 1.0000x reference)
"""Trainium2 Bass kernel for a single-layer transformer encoder
(pos-embed + causal/pad-masked MHA + 2x layernorm + relu FFN).

Contract: kernel(**inputs) takes the FULL unsharded inputs (as produced
by the problem's setup_inputs) and returns the FULL [16, 1024, 512] f32
output. Internally: data-parallel over the batch dim across 8
NeuronCores (2 batches per core), single SPMD NEFF.

Design notes (v3 -- software-pipelined across the 2 per-core batches):
 - All matmuls run in bf16 with f32 PSUM accumulation.
 - Scores computed transposed (S^T[k, q]); k-blocks processed in PAIRS
   sharing one 2-bank [128, 1024] PSUM tile so each softmax exp is a
   single wide ACT call. Causal mask for a diagonal pair is one
   [128, 1024] DVE multiply.
 - Softmax denominator: per-pair half-adds (Pool) + short DVE tree +
   ONE ones-matmul per (head, q-window) chain.
 - Emission is explicitly pipelined across batches: attention(b) is
   interleaved with Q/K projections(b+1), W_o(b) with V(b+1), so the
   static Tile schedule fills PE/ACT stalls of one batch with dense
   work of the next.
 - LN keeps ACT on Sqrt only between exp blocks (2 table sets, loads
   stay grouped); rstd via DVE reciprocal.
 - mean_k(V)/L for the pad-row fixup is precomputed on the host.
 - x = seq + pos_table precomputed on host in natural (f32) and
   transposed (bf16) layouts.
"""

import sys

for _p in ("/opt/trn_rl_repo",):
    if _p not in sys.path:
        sys.path.insert(0, _p)

import numpy as np
import ml_dtypes

import concourse.bass as bass
import concourse.tile as tile
from concourse import bacc, mybir
from concourse.bass_utils import run_bass_kernel_spmd

BF16 = ml_dtypes.bfloat16

N_CORES = 8
B = 16
L = 1024
D = 512
H = 8
DK = 128
BPC = B // N_CORES  # batches per core
LN_EPS = 1e-5
INV_TEMP = 1.0 / (np.sqrt(128.0) + 1e-6)

F32 = mybir.dt.float32
BF = mybir.dt.bfloat16

_cache = {}

import os
MMP_BUFS = int(os.environ.get("K_MMP_BUFS", "2"))
ACC_BUFS = int(os.environ.get("K_ACC_BUFS", "2"))
DN_BUFS = int(os.environ.get("K_DN_BUFS", "2"))
QT_BUFS = int(os.environ.get("K_QT_BUFS", "2"))
KT_BUFS = int(os.environ.get("K_KT_BUFS", "2"))
V_BUFS = int(os.environ.get("K_V_BUFS", "1"))
EXPS_BUFS = int(os.environ.get("K_EXPS_BUFS", "4"))
HS_BUFS = int(os.environ.get("K_HS_BUFS", "4"))


def _build(affine, use_b2, reps=1):
    """Build + compile the SPMD program. Returns nc."""
    nc = bacc.Bacc("TRN2", target_bir_lowering=False, debug=False,
                   num_devices=N_CORES)

    # ---- DRAM I/O ----
    xnat = nc.dram_tensor("xnat", [BPC, L, D], BF, kind="ExternalInput")
    xtr = nc.dram_tensor("xtr", [BPC, D, L], BF, kind="ExternalInput")
    padb = nc.dram_tensor("padb", [BPC, 1, L], mybir.dt.uint8, kind="ExternalInput")
    meanvt = nc.dram_tensor("meanvt", [BPC, 128, H], BF, kind="ExternalInput")
    wq = nc.dram_tensor("wq", [D, H * DK], BF, kind="ExternalInput")
    wk = nc.dram_tensor("wk", [D, H * DK], BF, kind="ExternalInput")
    wv = nc.dram_tensor("wv", [D, H * DK], BF, kind="ExternalInput")
    wo = nc.dram_tensor("wo", [H * DK, D], BF, kind="ExternalInput")
    w1t = nc.dram_tensor("w1t", [D, D], BF, kind="ExternalInput")
    w2t = nc.dram_tensor("w2t", [D, D], BF, kind="ExternalInput")
    b1c = nc.dram_tensor("b1c", [D, 1], F32, kind="ExternalInput")
    b2r = nc.dram_tensor("b2r", [1, D], F32, kind="ExternalInput")
    lng = nc.dram_tensor("lng", [1, D], F32, kind="ExternalInput")
    lnb = nc.dram_tensor("lnb", [1, D], F32, kind="ExternalInput")
    out = nc.dram_tensor("out", [BPC, L, D], F32, kind="ExternalOutput")

    # ---- inline constants ----
    tri_np = np.zeros((4, 128, 512), dtype=BF16)
    kk = np.arange(128)[:, None]
    qq = np.arange(512)[None, :]
    for r in range(4):
        tri_np[r] = (kk + 128 * r <= qq).astype(BF16)
    tri2_np = np.zeros((2, 128, 1024), dtype=BF16)
    tri2_np[0, :, 0:512] = tri_np[0]; tri2_np[0, :, 512:] = tri_np[1]
    tri2_np[1, :, 0:512] = tri_np[2]; tri2_np[1, :, 512:] = tri_np[3]
    tri2_d = nc.inline_tensor(tri2_np, name="tri2")
    ones_d = nc.inline_tensor(np.ones((128, 128), dtype=BF16), name="onesc")
    ident_d = nc.inline_tensor(np.eye(128, dtype=BF16), name="ident")

    def bcast_dram(ap2d, p=128):
        # [1, N] dram AP -> [p, N] partition-broadcast AP for DMA
        return bass.AP(tensor=ap2d.tensor, offset=ap2d.offset,
                       ap=[[0, p]] + list(ap2d.ap[1:]))

    Exp = mybir.ActivationFunctionType.Exp
    Sqrt = mybir.ActivationFunctionType.Sqrt
    mult = mybir.AluOpType.mult
    addop = mybir.AluOpType.add
    maxop = mybir.AluOpType.max

    with tile.TileContext(nc) as tc:
      with (
        tc.tile_pool(name="const", bufs=1) as cpool,
        tc.tile_pool(name="big", bufs=1) as bpool,
        tc.tile_pool(name="work", bufs=2) as wpool,
        tc.tile_pool(name="psum", bufs=2, space="PSUM") as pp,
      ):
        # ---- weights / constants ----
        wq_s = cpool.tile([128, 4, 1024], BF, name="wq_s")
        wk_s = cpool.tile([128, 4, 1024], BF, name="wk_s")
        wv_s = cpool.tile([128, 4, 1024], BF, name="wv_s")
        wo_s = cpool.tile([128, 8, 512], BF, name="wo_s")
        w1t_s = cpool.tile([128, 4, 512], BF, name="w1t_s")
        w2t_s = cpool.tile([128, 4, 512], BF, name="w2t_s")
        tri2_s = cpool.tile([128, 2, 1024], BF, name="tri2_s")
        ones_s = cpool.tile([128, 128], BF, name="ones_s")
        ident_s = cpool.tile([128, 128], BF, name="ident_s")
        b1c_s = cpool.tile([128, 4], F32, name="b1c_s")
        padb_s = cpool.tile([128, BPC, 1024], mybir.dt.uint8, name="padb_s")
        meanv_s = cpool.tile([128, BPC, H], BF, name="meanv_s")
        eps_s = cpool.tile([128, 1], F32, name="eps_s")
        nc.vector.memset(eps_s, LN_EPS)

        for dc in range(4):
            nc.sync.dma_start(out=wq_s[:, dc, :],
                              in_=wq.ap().rearrange("(c p) n -> p c n", p=128)[:, dc, :])
            nc.scalar.dma_start(out=wk_s[:, dc, :],
                                in_=wk.ap().rearrange("(c p) n -> p c n", p=128)[:, dc, :])
        nc.scalar.dma_start(out=wv_s, in_=wv.ap().rearrange("(c p) n -> p c n", p=128))
        nc.scalar.dma_start(out=ones_s, in_=ones_d.ap())
        nc.scalar.dma_start(out=tri2_s, in_=tri2_d.ap().rearrange("r p n -> p r n"))
        nc.scalar.dma_start(out=wo_s, in_=wo.ap().rearrange("(c p) n -> p c n", p=128))
        nc.scalar.dma_start(out=w1t_s, in_=w1t.ap().rearrange("(c p) n -> p c n", p=128))
        nc.scalar.dma_start(out=w2t_s, in_=w2t.ap().rearrange("(c p) n -> p c n", p=128))
        nc.scalar.dma_start(out=ident_s, in_=ident_d.ap())
        nc.scalar.dma_start(out=b1c_s, in_=b1c.ap().rearrange("(c p) one -> p (c one)", p=128))
        nc.gpsimd.dma_start(out=meanv_s, in_=meanvt.ap().rearrange("b p h -> p b h"))
        for b in range(BPC):
            nc.gpsimd.dma_start(out=padb_s[:, b, :], in_=bcast_dram(padb.ap()[b]))
        if use_b2:
            b2_s = cpool.tile([128, 512], F32, name="b2_s")
            nc.gpsimd.dma_start(out=b2_s, in_=bcast_dram(b2r.ap()))
        if affine:
            g_s = cpool.tile([128, 512], F32, name="g_s")
            bb_s = cpool.tile([128, 512], F32, name="bb_s")
            nc.gpsimd.dma_start(out=g_s, in_=bcast_dram(lng.ap()))
            nc.gpsimd.dma_start(out=bb_s, in_=bcast_dram(lnb.ap()))

        def layer_norm(dst, src, small):
            # dst = (src - mean) * rstd [* g + b]
            stats = small.tile([128, 6], F32, tag="lnstats", bufs=4)
            mv = small.tile([128, 2], F32, tag="lnmv", bufs=4)
            sd = small.tile([128, 1], F32, tag="lnsd", bufs=4)
            rs = small.tile([128, 1], F32, tag="lnrs", bufs=4)
            nmr = small.tile([128, 1], F32, tag="lnnmr", bufs=4)
            nc.vector.bn_stats(out=stats, in_=src)
            nc.vector.bn_aggr(out=mv, in_=stats)
            nc.scalar.activation(out=sd, in_=mv[:, 1:2], func=Sqrt, bias=eps_s)
            nc.vector.reciprocal(out=rs, in_=sd)
            nc.vector.scalar_tensor_tensor(out=nmr, in0=mv[:, 0:1], scalar=-1.0,
                                           in1=rs, op0=mult, op1=mult)
            nc.vector.tensor_scalar(out=dst, in0=src, scalar1=rs,
                                    scalar2=nmr, op0=mult, op1=addop)
            if affine:
                nc.vector.tensor_mul(dst, dst, g_s)
                nc.vector.tensor_add(dst, dst, bb_s)

        # ---------------- per-batch unit emitters ----------------
        class S:  # per-batch live tiles
            def __init__(self, b, uid):
                self.b = b
                self.uid = uid

        def alloc_attn_tiles(st):
            st.xT = bpool.tile([128, 4, 1024], BF, name=f"xT{st.uid}", tag="xT")
            st.qt = bpool.tile([128, 8, 1024], BF, name=f"qt{st.uid}", tag="QT", bufs=QT_BUFS)
            st.kt = bpool.tile([128, 8, 1024], BF, name=f"kt{st.uid}", tag="KT", bufs=KT_BUFS)
            st.v = bpool.tile([128, 8, 1024], BF, name=f"v{st.uid}", tag="V", bufs=V_BUFS)
            st.ctx = bpool.tile([128, 8, 1024], BF, name=f"ctx{st.uid}", tag="CTX")
            for dc in range(4):
                nc.sync.dma_start(out=st.xT[:, dc, :], in_=xtr.ap()[st.b].rearrange(
                    "(c p) n -> p c n", p=128)[:, dc, :])

        def qk_unit(st, u):
            # u in 0..15: (which, hc)
            which, hc = u % 2, u // 2
            w_s, dst = ((wq_s, st.qt) if which == 0 else (wk_s, st.kt))
            ps = pp.tile([128, 1024], F32, tag="mmp", bufs=MMP_BUFS)
            for jn in range(2):
                for dc in range(4):
                    nc.tensor.matmul(ps[:, jn * 512:(jn + 1) * 512],
                                     lhsT=w_s[:, dc, hc * 128:(hc + 1) * 128],
                                     rhs=st.xT[:, dc, jn * 512:(jn + 1) * 512],
                                     start=(dc == 0), stop=(dc == 3))
            nc.any.tensor_copy(dst[:, hc, :], ps)

        def v_unit(st, tt):
            ps = pp.tile([128, 1024], F32, tag="mmp", bufs=MMP_BUFS)
            for jn in range(2):
                for dc in range(4):
                    nc.tensor.matmul(ps[:, jn * 512:(jn + 1) * 512],
                                     lhsT=st.xT[:, dc, tt * 128:(tt + 1) * 128],
                                     rhs=wv_s[:, dc, jn * 512:(jn + 1) * 512],
                                     start=(dc == 0), stop=(dc == 3))
            nc.any.tensor_copy(st.v[:, tt, :], ps)

        def attn_chain(st, u):
            # u in 0..15: (h, j)
            h, j = u // 2, u % 2
            b = st.b
            hs = slice(h * 128, (h + 1) * 128)
            js = slice(j * 512, (j + 1) * 512)
            npair = 2 * (j + 1)
            cx_ps = pp.tile([128, 512], F32, tag="acc", bufs=ACC_BUFS)
            halves = []
            for p in range(npair):
                sc_ps = pp.tile([128, 1024], F32, tag="mmp", bufs=MMP_BUFS)
                nc.tensor.matmul(sc_ps[:, 0:512],
                                 lhsT=st.kt[:, h, (2 * p) * 128:(2 * p + 1) * 128],
                                 rhs=st.qt[:, h, js],
                                 start=True, stop=True)
                nc.tensor.matmul(sc_ps[:, 512:1024],
                                 lhsT=st.kt[:, h, (2 * p + 1) * 128:(2 * p + 2) * 128],
                                 rhs=st.qt[:, h, js],
                                 start=True, stop=True)
                ex = wpool.tile([128, 1024], BF, tag="expS", bufs=EXPS_BUFS)
                nc.scalar.activation(out=ex, in_=sc_ps, func=Exp,
                                     scale=INV_TEMP)
                r0 = 2 * p - 4 * j
                if r0 >= 0:
                    nc.vector.tensor_mul(ex, ex, tri2_s[:, r0 // 2, :])
                nc.tensor.matmul(cx_ps, lhsT=st.v[:, 2 * p, hs],
                                 rhs=ex[:, 0:512],
                                 start=(p == 0), stop=False)
                nc.tensor.matmul(cx_ps, lhsT=st.v[:, 2 * p + 1, hs],
                                 rhs=ex[:, 512:1024],
                                 start=False, stop=(p == npair - 1))
                hsum = wpool.tile([128, 512], BF, tag="hsum", bufs=HS_BUFS)
                nc.gpsimd.tensor_add(hsum, ex[:, 0:512], ex[:, 512:1024])
                halves.append(hsum)
            # denominator tree on DVE, then ONE ones-matmul
            nc.vector.tensor_add(halves[0], halves[0], halves[1])
            if npair == 4:
                nc.vector.tensor_add(halves[2], halves[2], halves[3])
                nc.vector.tensor_add(halves[0], halves[0], halves[2])
            dn_ps = pp.tile([128, 512], F32, tag="dn", bufs=DN_BUFS)
            nc.tensor.matmul(dn_ps, lhsT=ones_s, rhs=halves[0],
                             start=True, stop=True)
            rcp = wpool.tile([128, 512], F32, tag="rcp", bufs=2)
            nc.vector.reciprocal(out=rcp, in_=dn_ps)
            nc.vector.tensor_mul(st.ctx[:, h, js], cx_ps, rcp)
            nc.vector.copy_predicated(
                out=st.ctx[:, h, js],
                mask=padb_s[:, b, js],
                data=meanv_s[:, b, h:h + 1].to_broadcast([128, 512]))

        def alloc_post_tiles(st):
            st.Xn = bpool.tile([128, 8, 512], BF, name=f"Xn{st.uid}", tag="Xn")
            st.xt_sb = bpool.tile([128, 4, 1024], BF, name=f"xts{st.uid}", tag="XT")
            st.rel = bpool.tile([128, 4, 1024], BF, name=f"rel{st.uid}", tag="REL")

        def wo_unit(st, qt):
            qs = slice(qt * 128, (qt + 1) * 128)
            xn_t = wpool.tile([128, 512], BF, tag="xn", bufs=2)
            nc.scalar.dma_start(out=xn_t, in_=xnat.ap()[st.b].rearrange(
                "(c p) n -> p c n", p=128)[:, qt, :])
            va_ps = pp.tile([128, 512], F32, tag="acc", bufs=ACC_BUFS)
            # residual x rides the accumulation as an identity matmul
            nc.tensor.matmul(va_ps, lhsT=ident_s, rhs=xn_t,
                             start=True, stop=False)
            for h in range(8):
                nc.tensor.matmul(va_ps, lhsT=st.ctx[:, h, qs],
                                 rhs=wo_s[:, h, :],
                                 start=False, stop=(h == 7))
            layer_norm(st.Xn[:, qt, :], va_ps, wpool)
            tp_ps = pp.tile([128, 512], BF, tag="dn", bufs=DN_BUFS)
            for dc in range(4):
                nc.tensor.transpose(tp_ps[:, dc * 128:(dc + 1) * 128],
                                    st.Xn[:, qt, dc * 128:(dc + 1) * 128],
                                    ident_s)
            nc.any.tensor_copy(
                st.xt_sb[:, :, qt * 128:(qt + 1) * 128],
                tp_ps.rearrange("p (c n) -> p c n", c=4))

        def ffn_unit(st, u):
            # u in 0..7: (j, fc)
            j, fc = u // 4, u % 4
            js = slice(j * 512, (j + 1) * 512)
            f_ps = pp.tile([128, 512], F32, tag="acc", bufs=ACC_BUFS)
            for dc in range(4):
                nc.tensor.matmul(f_ps,
                                 lhsT=w1t_s[:, dc, fc * 128:(fc + 1) * 128],
                                 rhs=st.xt_sb[:, dc, js],
                                 start=(dc == 0), stop=(dc == 3))
            nc.vector.tensor_scalar(out=st.rel[:, fc, js], in0=f_ps,
                                    scalar1=b1c_s[:, fc:fc + 1],
                                    scalar2=0.0, op0=addop, op1=maxop)

        def s7_unit(st, qt):
            qs = slice(qt * 128, (qt + 1) * 128)
            ff_ps = pp.tile([128, 512], F32, tag="acc", bufs=ACC_BUFS)
            # residual X rides the accumulation as an identity matmul
            nc.tensor.matmul(ff_ps, lhsT=ident_s, rhs=st.Xn[:, qt, :],
                             start=True, stop=False)
            for fc in range(4):
                nc.tensor.matmul(ff_ps, lhsT=st.rel[:, fc, qs],
                                 rhs=w2t_s[:, fc, :],
                                 start=False, stop=(fc == 3))
            if use_b2:
                nc.vector.tensor_add(ff_ps, ff_ps, b2_s)
            o_t = wpool.tile([128, 512], F32, tag="out", bufs=2)
            layer_norm(o_t, ff_ps, wpool)
            nc.sync.dma_start(out=out.ap()[st.b, qt * 128:(qt + 1) * 128, :],
                              in_=o_t)

        # ---------------- pipelined emission over the batch stream ----------
        stream = [S(b, f"r{r}b{b}") for r in range(reps) for b in range(BPC)]
        cur = stream[0]
        alloc_attn_tiles(cur)
        for u in range(16):
            qk_unit(cur, u)
        for tt in range(8):
            v_unit(cur, tt)
        for i, cur in enumerate(stream):
            nxt = stream[i + 1] if i + 1 < len(stream) else None
            if nxt is not None:
                alloc_attn_tiles(nxt)
            # attention(cur) || QK(nxt)
            for u in range(16):
                attn_chain(cur, u)
                if nxt is not None:
                    qk_unit(nxt, u)
            # W_o/LN1(cur) || V(nxt)
            alloc_post_tiles(cur)
            for qt in range(8):
                wo_unit(cur, qt)
                if nxt is not None:
                    v_unit(nxt, qt)
            # FFN + out (attention(nxt) gets pulled into the gaps)
            for u in range(8):
                ffn_unit(cur, u)
            for qt in range(8):
                s7_unit(cur, qt)

    nc.compile()
    return nc


def _get_nc(affine, use_b2, reps=1):
    key = (affine, use_b2, reps)
    if key not in _cache:
        _cache[key] = _build(affine, use_b2, reps)
    return _cache[key]


def _prep_inputs(seq_h, pad_mask, pos_table, W_q, W_k, W_v, W_o, w1, b1, w2,
                 b2, ln_g, ln_b):
    seq_h = np.asarray(seq_h, dtype=np.float32)
    pad_mask = np.asarray(pad_mask)
    affine = not (np.all(np.asarray(ln_g) == 1.0) and np.all(np.asarray(ln_b) == 0.0))
    use_b2 = bool(np.any(np.asarray(b2) != 0.0))

    common = {
        "wq": np.asarray(W_q, np.float32).astype(BF16),
        "wk": np.asarray(W_k, np.float32).astype(BF16),
        "wv": np.asarray(W_v, np.float32).astype(BF16),
        "wo": np.asarray(W_o, np.float32).astype(BF16),
        "w1t": np.ascontiguousarray(np.asarray(w1, np.float32).T).astype(BF16),
        "w2t": np.ascontiguousarray(np.asarray(w2, np.float32).T).astype(BF16),
        "b1c": np.asarray(b1, np.float32).reshape(D, 1),
        "b2r": np.asarray(b2, np.float32).reshape(1, D),
        "lng": np.asarray(ln_g, np.float32).reshape(1, D),
        "lnb": np.asarray(ln_b, np.float32).reshape(1, D),
    }
    x = seq_h + np.asarray(pos_table, np.float32)[:L][None]
    xnat_bf = x.astype(BF16)
    xT = np.ascontiguousarray(x.transpose(0, 2, 1)).astype(BF16)
    padb = pad_mask.astype(np.uint8).reshape(B, 1, L)
    # mean_k V / L per head, laid out [dv-in-head, head]
    mv = (x.sum(axis=1) @ np.asarray(W_v, np.float32)) / L  # [B, H*DK]
    meanvt = np.ascontiguousarray(
        mv.reshape(B, H, DK).transpose(0, 2, 1)).astype(BF16)  # [B, 128, H]

    in_maps = []
    for c in range(N_CORES):
        sl = slice(c * BPC, (c + 1) * BPC)
        m = dict(common)
        m["xnat"] = np.ascontiguousarray(xnat_bf[sl])
        m["xtr"] = np.ascontiguousarray(xT[sl])
        m["padb"] = np.ascontiguousarray(padb[sl])
        m["meanvt"] = np.ascontiguousarray(meanvt[sl])
        in_maps.append(m)
    return in_maps, affine, use_b2


def kernel(**inputs) -> np.ndarray:
    in_maps, affine, use_b2 = _prep_inputs(**inputs)
    nc = _get_nc(affine, use_b2)
    res = run_bass_kernel_spmd(nc, in_maps, core_ids=list(range(N_CORES)))
    return np.concatenate([np.asarray(r["out"]) for r in res.results], axis=0)


# revision 28
# speedup vs baseline: 1.0606x; 1.0606x over previous
"""Trainium2 Bass kernel for a single-layer transformer encoder
(pos-embed + causal/pad-masked MHA + 2x layernorm + relu FFN).

Contract: kernel(**inputs) takes the FULL unsharded inputs (as produced
by the problem's setup_inputs) and returns the FULL [16, 1024, 512] f32
output. Internally: data-parallel over the batch dim across 8
NeuronCores (2 batches per core), single SPMD NEFF.

Design notes (v3 -- software-pipelined across the 2 per-core batches):
 - All matmuls run in bf16 with f32 PSUM accumulation.
 - Scores computed transposed (S^T[k, q]); k-blocks processed in PAIRS
   sharing one 2-bank [128, 1024] PSUM tile so each softmax exp is a
   single wide ACT call. Causal mask for a diagonal pair is one
   [128, 1024] DVE multiply.
 - Softmax denominator: per-pair half-adds (Pool) + short DVE tree +
   ONE ones-matmul per (head, q-window) chain.
 - Emission is explicitly pipelined across batches: attention(b) is
   interleaved with Q/K projections(b+1), W_o(b) with V(b+1), so the
   static Tile schedule fills PE/ACT stalls of one batch with dense
   work of the next.
 - LN keeps ACT on Sqrt only between exp blocks (2 table sets, loads
   stay grouped); rstd via DVE reciprocal.
 - mean_k(V)/L for the pad-row fixup is precomputed on the host.
 - x = seq + pos_table precomputed on host in natural (f32) and
   transposed (bf16) layouts.
"""

import sys

for _p in ("/opt/trn_rl_repo",):
    if _p not in sys.path:
        sys.path.insert(0, _p)

import numpy as np
import ml_dtypes

import concourse.bass as bass
import concourse.tile as tile
from concourse import bacc, mybir
from concourse.bass_utils import run_bass_kernel_spmd

BF16 = ml_dtypes.bfloat16
E4M3 = ml_dtypes.float8_e4m3

N_CORES = 8
B = 16
L = 1024
D = 512
H = 8
DK = 128
BPC = B // N_CORES  # batches per core
LN_EPS = 1e-5
INV_TEMP = 1.0 / (np.sqrt(128.0) + 1e-6)

F32 = mybir.dt.float32
BF = mybir.dt.bfloat16
FP8 = mybir.dt.float8e4

_cache = {}

import os
MMP_BUFS = int(os.environ.get("K_MMP_BUFS", "2"))
ACC_BUFS = int(os.environ.get("K_ACC_BUFS", "2"))
DN_BUFS = int(os.environ.get("K_DN_BUFS", "2"))
QT_BUFS = int(os.environ.get("K_QT_BUFS", "2"))
KT_BUFS = int(os.environ.get("K_KT_BUFS", "2"))
V_BUFS = int(os.environ.get("K_V_BUFS", "1"))
EXPS_BUFS = int(os.environ.get("K_EXPS_BUFS", "4"))
HS_BUFS = int(os.environ.get("K_HS_BUFS", "4"))


def _build(affine, use_b2, reps=1):
    """Build + compile the SPMD program. Returns nc."""
    nc = bacc.Bacc("TRN2", target_bir_lowering=False, debug=False,
                   num_devices=N_CORES)

    # ---- DRAM I/O ----
    xnat = nc.dram_tensor("xnat", [BPC, L, D], BF, kind="ExternalInput")
    xtr = nc.dram_tensor("xtr", [BPC, D, L], BF, kind="ExternalInput")
    padb = nc.dram_tensor("padb", [BPC, 1, L], mybir.dt.uint8, kind="ExternalInput")
    meanvt = nc.dram_tensor("meanvt", [BPC, 128, H], FP8, kind="ExternalInput")
    wq = nc.dram_tensor("wq", [D, H * DK], BF, kind="ExternalInput")
    wk = nc.dram_tensor("wk", [D, H * DK], BF, kind="ExternalInput")
    wv = nc.dram_tensor("wv", [D, H * DK], BF, kind="ExternalInput")
    wo = nc.dram_tensor("wo", [H * DK, D], FP8, kind="ExternalInput")
    w1t = nc.dram_tensor("w1t", [D, D], BF, kind="ExternalInput")
    w2t = nc.dram_tensor("w2t", [D, D], BF, kind="ExternalInput")
    b1c = nc.dram_tensor("b1c", [D, 1], F32, kind="ExternalInput")
    b2r = nc.dram_tensor("b2r", [1, D], F32, kind="ExternalInput")
    lng = nc.dram_tensor("lng", [1, D], F32, kind="ExternalInput")
    lnb = nc.dram_tensor("lnb", [1, D], F32, kind="ExternalInput")
    out = nc.dram_tensor("out", [BPC, L, D], F32, kind="ExternalOutput")

    # ---- inline constants ----
    tri_np = np.zeros((4, 128, 512), dtype=BF16)
    kk = np.arange(128)[:, None]
    qq = np.arange(512)[None, :]
    for r in range(4):
        tri_np[r] = (kk + 128 * r <= qq).astype(BF16)
    tri2_np = np.zeros((2, 128, 1024), dtype=BF16)
    tri2_np[0, :, 0:512] = tri_np[0]; tri2_np[0, :, 512:] = tri_np[1]
    tri2_np[1, :, 0:512] = tri_np[2]; tri2_np[1, :, 512:] = tri_np[3]
    tri2_d = nc.inline_tensor(tri2_np, name="tri2")
    ones_d = nc.inline_tensor(np.ones((128, 128), dtype=BF16), name="onesc")
    ident_d = nc.inline_tensor(np.eye(128, dtype=BF16), name="ident")

    def bcast_dram(ap2d, p=128):
        # [1, N] dram AP -> [p, N] partition-broadcast AP for DMA
        return bass.AP(tensor=ap2d.tensor, offset=ap2d.offset,
                       ap=[[0, p]] + list(ap2d.ap[1:]))

    Exp = mybir.ActivationFunctionType.Exp
    Sqrt = mybir.ActivationFunctionType.Sqrt
    mult = mybir.AluOpType.mult
    addop = mybir.AluOpType.add
    maxop = mybir.AluOpType.max

    with tile.TileContext(nc) as tc:
      with (
        tc.tile_pool(name="const", bufs=1) as cpool,
        tc.tile_pool(name="big", bufs=1) as bpool,
        tc.tile_pool(name="work", bufs=2) as wpool,
        tc.tile_pool(name="psum", bufs=2, space="PSUM") as pp,
      ):
        # ---- weights / constants ----
        wq_s = cpool.tile([128, 4, 1024], BF, name="wq_s")
        wk_s = cpool.tile([128, 4, 1024], BF, name="wk_s")
        wv_s = cpool.tile([128, 4, 1024], BF, name="wv_s")
        wo_s = cpool.tile([128, 8, 512], FP8, name="wo_s")
        w1t_s = cpool.tile([128, 4, 512], BF, name="w1t_s")
        w2t_s = cpool.tile([128, 4, 512], BF, name="w2t_s")
        tri2_s = cpool.tile([128, 2, 1024], BF, name="tri2_s")
        ones_s = cpool.tile([128, 128], BF, name="ones_s")
        ident_s = cpool.tile([128, 128], BF, name="ident_s")
        b1c_s = cpool.tile([128, 4], F32, name="b1c_s")
        padb_s = cpool.tile([128, BPC, 1024], mybir.dt.uint8, name="padb_s")
        meanv_s = cpool.tile([128, BPC, H], FP8, name="meanv_s")
        eps_s = cpool.tile([128, 1], F32, name="eps_s")
        nc.vector.memset(eps_s, LN_EPS)

        for dc in range(4):
            nc.sync.dma_start(out=wq_s[:, dc, :],
                              in_=wq.ap().rearrange("(c p) n -> p c n", p=128)[:, dc, :])
            nc.scalar.dma_start(out=wk_s[:, dc, :],
                                in_=wk.ap().rearrange("(c p) n -> p c n", p=128)[:, dc, :])
        nc.scalar.dma_start(out=wv_s, in_=wv.ap().rearrange("(c p) n -> p c n", p=128))
        nc.scalar.dma_start(out=ones_s, in_=ones_d.ap())
        nc.scalar.dma_start(out=tri2_s, in_=tri2_d.ap().rearrange("r p n -> p r n"))
        nc.scalar.dma_start(out=wo_s, in_=wo.ap().rearrange("(c p) n -> p c n", p=128))
        nc.scalar.dma_start(out=w1t_s, in_=w1t.ap().rearrange("(c p) n -> p c n", p=128))
        nc.scalar.dma_start(out=w2t_s, in_=w2t.ap().rearrange("(c p) n -> p c n", p=128))
        nc.scalar.dma_start(out=ident_s, in_=ident_d.ap())
        nc.scalar.dma_start(out=b1c_s, in_=b1c.ap().rearrange("(c p) one -> p (c one)", p=128))
        nc.gpsimd.dma_start(out=meanv_s, in_=meanvt.ap().rearrange("b p h -> p b h"))
        for b in range(BPC):
            nc.gpsimd.dma_start(out=padb_s[:, b, :], in_=bcast_dram(padb.ap()[b]))
        if use_b2:
            b2_s = cpool.tile([128, 512], F32, name="b2_s")
            nc.gpsimd.dma_start(out=b2_s, in_=bcast_dram(b2r.ap()))
        if affine:
            g_s = cpool.tile([128, 512], F32, name="g_s")
            bb_s = cpool.tile([128, 512], F32, name="bb_s")
            nc.gpsimd.dma_start(out=g_s, in_=bcast_dram(lng.ap()))
            nc.gpsimd.dma_start(out=bb_s, in_=bcast_dram(lnb.ap()))

        def layer_norm(dst, src, small):
            # dst = (src - mean) * rstd [* g + b]
            stats = small.tile([128, 6], F32, tag="lnstats", bufs=4)
            mv = small.tile([128, 2], F32, tag="lnmv", bufs=4)
            sd = small.tile([128, 1], F32, tag="lnsd", bufs=4)
            rs = small.tile([128, 1], F32, tag="lnrs", bufs=4)
            nmr = small.tile([128, 1], F32, tag="lnnmr", bufs=4)
            nc.vector.bn_stats(out=stats, in_=src)
            nc.vector.bn_aggr(out=mv, in_=stats)
            nc.scalar.activation(out=sd, in_=mv[:, 1:2], func=Sqrt, bias=eps_s)
            nc.vector.reciprocal(out=rs, in_=sd)
            nc.vector.scalar_tensor_tensor(out=nmr, in0=mv[:, 0:1], scalar=-1.0,
                                           in1=rs, op0=mult, op1=mult)
            nc.vector.tensor_scalar(out=dst, in0=src, scalar1=rs,
                                    scalar2=nmr, op0=mult, op1=addop)
            if affine:
                nc.vector.tensor_mul(dst, dst, g_s)
                nc.vector.tensor_add(dst, dst, bb_s)

        # ---------------- per-batch unit emitters ----------------
        class S:  # per-batch live tiles
            def __init__(self, b, uid):
                self.b = b
                self.uid = uid

        def alloc_attn_tiles(st):
            st.xT = bpool.tile([128, 4, 1024], BF, name=f"xT{st.uid}", tag="xT")
            st.qt = bpool.tile([128, 8, 1024], BF, name=f"qt{st.uid}", tag="QT", bufs=QT_BUFS)
            st.kt = bpool.tile([128, 8, 1024], BF, name=f"kt{st.uid}", tag="KT", bufs=KT_BUFS)
            st.v = bpool.tile([128, 8, 1024], BF, name=f"v{st.uid}", tag="V", bufs=V_BUFS)
            st.ctx = bpool.tile([128, 8, 1024], FP8, name=f"ctx{st.uid}", tag="CTX")
            for dc in range(4):
                nc.sync.dma_start(out=st.xT[:, dc, :], in_=xtr.ap()[st.b].rearrange(
                    "(c p) n -> p c n", p=128)[:, dc, :])

        def qk_unit(st, u):
            # u in 0..15: (which, hc)
            which, hc = u % 2, u // 2
            w_s, dst = ((wq_s, st.qt) if which == 0 else (wk_s, st.kt))
            ps = pp.tile([128, 1024], F32, tag="mmp", bufs=MMP_BUFS)
            for jn in range(2):
                for dc in range(4):
                    nc.tensor.matmul(ps[:, jn * 512:(jn + 1) * 512],
                                     lhsT=w_s[:, dc, hc * 128:(hc + 1) * 128],
                                     rhs=st.xT[:, dc, jn * 512:(jn + 1) * 512],
                                     start=(dc == 0), stop=(dc == 3))
            nc.any.tensor_copy(dst[:, hc, :], ps)

        def v_unit(st, tt):
            ps = pp.tile([128, 1024], F32, tag="mmp", bufs=MMP_BUFS)
            for jn in range(2):
                for dc in range(4):
                    nc.tensor.matmul(ps[:, jn * 512:(jn + 1) * 512],
                                     lhsT=st.xT[:, dc, tt * 128:(tt + 1) * 128],
                                     rhs=wv_s[:, dc, jn * 512:(jn + 1) * 512],
                                     start=(dc == 0), stop=(dc == 3))
            nc.any.tensor_copy(st.v[:, tt, :], ps)

        def attn_chain(st, u):
            # u in 0..15: (h, j)
            h, j = u // 2, u % 2
            b = st.b
            hs = slice(h * 128, (h + 1) * 128)
            js = slice(j * 512, (j + 1) * 512)
            npair = 2 * (j + 1)
            cx_ps = pp.tile([128, 512], F32, tag="acc", bufs=ACC_BUFS)
            halves = []
            for p in range(npair):
                sc_ps = pp.tile([128, 1024], F32, tag="mmp", bufs=MMP_BUFS)
                nc.tensor.matmul(sc_ps[:, 0:512],
                                 lhsT=st.kt[:, h, (2 * p) * 128:(2 * p + 1) * 128],
                                 rhs=st.qt[:, h, js],
                                 start=True, stop=True)
                nc.tensor.matmul(sc_ps[:, 512:1024],
                                 lhsT=st.kt[:, h, (2 * p + 1) * 128:(2 * p + 2) * 128],
                                 rhs=st.qt[:, h, js],
                                 start=True, stop=True)
                ex = wpool.tile([128, 1024], BF, tag="expS", bufs=EXPS_BUFS)
                nc.scalar.activation(out=ex, in_=sc_ps, func=Exp,
                                     scale=INV_TEMP)
                r0 = 2 * p - 4 * j
                if r0 >= 0:
                    nc.vector.tensor_mul(ex, ex, tri2_s[:, r0 // 2, :])
                nc.tensor.matmul(cx_ps, lhsT=st.v[:, 2 * p, hs],
                                 rhs=ex[:, 0:512],
                                 start=(p == 0), stop=False)
                nc.tensor.matmul(cx_ps, lhsT=st.v[:, 2 * p + 1, hs],
                                 rhs=ex[:, 512:1024],
                                 start=False, stop=(p == npair - 1))
                hsum = wpool.tile([128, 512], BF, tag="hsum", bufs=HS_BUFS)
                nc.gpsimd.tensor_add(hsum, ex[:, 0:512], ex[:, 512:1024])
                halves.append(hsum)
            # denominator tree on DVE, then ONE ones-matmul
            nc.vector.tensor_add(halves[0], halves[0], halves[1])
            if npair == 4:
                nc.vector.tensor_add(halves[2], halves[2], halves[3])
                nc.vector.tensor_add(halves[0], halves[0], halves[2])
            dn_ps = pp.tile([128, 512], F32, tag="dn", bufs=DN_BUFS)
            nc.tensor.matmul(dn_ps, lhsT=ones_s, rhs=halves[0],
                             start=True, stop=True)
            rcp = wpool.tile([128, 512], F32, tag="rcp", bufs=2)
            nc.vector.reciprocal(out=rcp, in_=dn_ps)
            nc.vector.tensor_mul(st.ctx[:, h, js], cx_ps, rcp)
            nc.vector.copy_predicated(
                out=st.ctx[:, h, js],
                mask=padb_s[:, b, js],
                data=meanv_s[:, b, h:h + 1].to_broadcast([128, 512]))

        def alloc_post_tiles(st):
            st.Xn = bpool.tile([128, 8, 512], BF, name=f"Xn{st.uid}", tag="Xn")
            st.xt_sb = bpool.tile([128, 4, 1024], BF, name=f"xts{st.uid}", tag="XT")
            st.rel = bpool.tile([128, 4, 1024], BF, name=f"rel{st.uid}", tag="REL")

        def wo_unit(st, qt):
            qs = slice(qt * 128, (qt + 1) * 128)
            xn_t = wpool.tile([128, 512], BF, tag="xn", bufs=2)
            nc.scalar.dma_start(out=xn_t, in_=xnat.ap()[st.b].rearrange(
                "(c p) n -> p c n", p=128)[:, qt, :])
            va_ps = pp.tile([128, 512], F32, tag="acc", bufs=ACC_BUFS)
            # residual x rides the accumulation as an identity matmul
            nc.tensor.matmul(va_ps, lhsT=ident_s, rhs=xn_t,
                             start=True, stop=False)
            # fp8 DoubleRow: contract two heads (2x128 dv) per matmul
            for hp in range(4):
                nc.tensor.matmul(va_ps, lhsT=st.ctx[:, 2 * hp:2 * hp + 2, qs],
                                 rhs=wo_s[:, 2 * hp:2 * hp + 2, :],
                                 perf_mode=mybir.MatmulPerfMode.DoubleRow,
                                 start=False, stop=(hp == 3))
            layer_norm(st.Xn[:, qt, :], va_ps, wpool)
            tp_ps = pp.tile([128, 512], BF, tag="dn", bufs=DN_BUFS)
            for dc in range(4):
                nc.tensor.transpose(tp_ps[:, dc * 128:(dc + 1) * 128],
                                    st.Xn[:, qt, dc * 128:(dc + 1) * 128],
                                    ident_s)
            nc.any.tensor_copy(
                st.xt_sb[:, :, qt * 128:(qt + 1) * 128],
                tp_ps.rearrange("p (c n) -> p c n", c=4))

        def ffn_unit(st, u):
            # u in 0..7: (j, fc)
            j, fc = u // 4, u % 4
            js = slice(j * 512, (j + 1) * 512)
            f_ps = pp.tile([128, 512], F32, tag="acc", bufs=ACC_BUFS)
            for dc in range(4):
                nc.tensor.matmul(f_ps,
                                 lhsT=w1t_s[:, dc, fc * 128:(fc + 1) * 128],
                                 rhs=st.xt_sb[:, dc, js],
                                 start=(dc == 0), stop=(dc == 3))
            nc.vector.tensor_scalar(out=st.rel[:, fc, js], in0=f_ps,
                                    scalar1=b1c_s[:, fc:fc + 1],
                                    scalar2=0.0, op0=addop, op1=maxop)

        def s7_unit(st, qt):
            qs = slice(qt * 128, (qt + 1) * 128)
            ff_ps = pp.tile([128, 512], F32, tag="acc", bufs=ACC_BUFS)
            # residual X rides the accumulation as an identity matmul
            nc.tensor.matmul(ff_ps, lhsT=ident_s, rhs=st.Xn[:, qt, :],
                             start=True, stop=False)
            for fc in range(4):
                nc.tensor.matmul(ff_ps, lhsT=st.rel[:, fc, qs],
                                 rhs=w2t_s[:, fc, :],
                                 start=False, stop=(fc == 3))
            if use_b2:
                nc.vector.tensor_add(ff_ps, ff_ps, b2_s)
            o_t = wpool.tile([128, 512], F32, tag="out", bufs=2)
            layer_norm(o_t, ff_ps, wpool)
            nc.sync.dma_start(out=out.ap()[st.b, qt * 128:(qt + 1) * 128, :],
                              in_=o_t)

        # ---------------- pipelined emission over the batch stream ----------
        stream = [S(b, f"r{r}b{b}") for r in range(reps) for b in range(BPC)]
        cur = stream[0]
        alloc_attn_tiles(cur)
        for u in range(16):
            qk_unit(cur, u)
        for tt in range(8):
            v_unit(cur, tt)
        for i, cur in enumerate(stream):
            nxt = stream[i + 1] if i + 1 < len(stream) else None
            if nxt is not None:
                alloc_attn_tiles(nxt)
            # attention(cur) || QK(nxt)
            for u in range(16):
                attn_chain(cur, u)
                if nxt is not None:
                    qk_unit(nxt, u)
            # W_o/LN1(cur) || V(nxt)
            alloc_post_tiles(cur)
            for qt in range(8):
                wo_unit(cur, qt)
                if nxt is not None:
                    v_unit(nxt, qt)
            # FFN + out (attention(nxt) gets pulled into the gaps)
            for u in range(8):
                ffn_unit(cur, u)
            for qt in range(8):
                s7_unit(cur, qt)

    nc.compile()
    return nc


def _get_nc(affine, use_b2, reps=1):
    key = (affine, use_b2, reps)
    if key not in _cache:
        _cache[key] = _build(affine, use_b2, reps)
    return _cache[key]


def _prep_inputs(seq_h, pad_mask, pos_table, W_q, W_k, W_v, W_o, w1, b1, w2,
                 b2, ln_g, ln_b):
    seq_h = np.asarray(seq_h, dtype=np.float32)
    pad_mask = np.asarray(pad_mask)
    affine = not (np.all(np.asarray(ln_g) == 1.0) and np.all(np.asarray(ln_b) == 0.0))
    use_b2 = bool(np.any(np.asarray(b2) != 0.0))

    common = {
        "wq": np.asarray(W_q, np.float32).astype(BF16),
        "wk": np.asarray(W_k, np.float32).astype(BF16),
        "wv": np.asarray(W_v, np.float32).astype(BF16),
        "wo": np.asarray(W_o, np.float32).astype(E4M3),
        "w1t": np.ascontiguousarray(np.asarray(w1, np.float32).T).astype(BF16),
        "w2t": np.ascontiguousarray(np.asarray(w2, np.float32).T).astype(BF16),
        "b1c": np.asarray(b1, np.float32).reshape(D, 1),
        "b2r": np.asarray(b2, np.float32).reshape(1, D),
        "lng": np.asarray(ln_g, np.float32).reshape(1, D),
        "lnb": np.asarray(ln_b, np.float32).reshape(1, D),
    }
    x = seq_h + np.asarray(pos_table, np.float32)[:L][None]
    xnat_bf = x.astype(BF16)
    xT = np.ascontiguousarray(x.transpose(0, 2, 1)).astype(BF16)
    padb = pad_mask.astype(np.uint8).reshape(B, 1, L)
    # mean_k V / L per head, laid out [dv-in-head, head]
    mv = (x.sum(axis=1) @ np.asarray(W_v, np.float32)) / L  # [B, H*DK]
    meanvt = np.ascontiguousarray(
        mv.reshape(B, H, DK).transpose(0, 2, 1)).astype(E4M3)  # [B, 128, H]

    in_maps = []
    for c in range(N_CORES):
        sl = slice(c * BPC, (c + 1) * BPC)
        m = dict(common)
        m["xnat"] = np.ascontiguousarray(xnat_bf[sl])
        m["xtr"] = np.ascontiguousarray(xT[sl])
        m["padb"] = np.ascontiguousarray(padb[sl])
        m["meanvt"] = np.ascontiguousarray(meanvt[sl])
        in_maps.append(m)
    return in_maps, affine, use_b2


def kernel(**inputs) -> np.ndarray:
    in_maps, affine, use_b2 = _prep_inputs(**inputs)
    nc = _get_nc(affine, use_b2)
    res = run_bass_kernel_spmd(nc, in_maps, core_ids=list(range(N_CORES)))
    return np.concatenate([np.asarray(r["out"]) for r in res.results], axis=0)
